# revision 53
# baseline (speedup 1.0000x reference)
"""Dense dot-product attention on 8 Trainium2 NeuronCores.

Problem: query/key/value [32, 2048, 64] fp32 -> softmax(Q K^T / 8) V.
Sharding: batch dim split 4-per-core across 8 cores (data parallel, no
collectives). Each core computes full attention for its 4 batches.

v3 design: the PE does ONLY matmuls (no transposes); softmax exp is
split across ScalarE (true exp, even k-tiles) and the DVE (two-phase
Schraudolph bit-trick exp, odd k-tiles); all transposes ride the DMA
xbar.

Per-batch dataflow:
  1. DMA Q,K,V natural [2048,64] fp32 -> stage [128,16,64]; gpsimd
     casts to fp16 (Q,K natural; V into vr [128,16,68] with a ones
     column at 64 = PV denominator trick; vr_sq2 = sqrt(2)*vr).
  2. One xbar-transpose DMA for Q and K: [128,1024] fp16 ->
     [128,8,128] where out[p,m,c] = in[c, m*128+p]: partitions 0:64
     hold even k-tiles transposed, 64:128 odd k-tiles.  K is used in
     this paired layout directly; Q is unscrambled + duplicated into
     qfull [128, 8, 2, 128] (every q-tile on both partition halves).
  3. Scores: per k-pair, 2x2 matmuls with contraction 64 on opposite
     PE row-groups (tile_position (0,0)/(64,0) via base partitions);
     fp32 PSUM s_a (even tile) / s_b (odd tile) [128, 1024].
  4. exp (scale 1/8 folded): ScalarE activation on s_a -> e_a fp16.
     DVE on s_b: e_b1 = bitcast16(i16(A*x + B)) approximates exp/2
     with a +-3% log-linear sawtooth; e_b2 = e_b1 - 512 (bits) is the
     half-octave phase-shifted variant scaled 1/(2*sqrt2).  The PV
     matmul sums  vr^T e_b1 + (sqrt2 vr)^T e_b2  = exp * (1 + mean of
     the two sawtooth phases): ~0.45% rms residual, and softmax
     normalization cancels the common-mode bias.
  5. PV: lhsT = [V | 1] natural [128, 65] fp16 -> out^T [65, q] fp32
     PSUM accumulated over 16 k-tiles (6 matmuls per k-pair); row 64
     is the softmax denominator (the sqrt2 ones-column keeps the b2
     stream's denominator share consistent).
  6. ScalarE evacuates pv -> fp16 SBUF; xbar-transpose (issued from
     the scalar engine's DMA queue) -> o_t [128, 8, 96] with
     q = m*128+p; DVE reciprocal of col 64 + one broadcast
     tensor_tensor multiply normalizes -> o_all fp32; DMA store.
     The reciprocal/normalize/store for each q-half are emitted
     deferred, woven into the next q-half's instruction stream, so no
     engine queue head-of-line blocks on the output path.
"""

import numpy as np

B, L, D = 32, 2048, 64
NCORES = 8
B_SH = B // NCORES          # 4 batches per core
LT = L // 128               # 16 k/q tiles of 128
NQH = 2                     # q processed in halves of 1024
QHW = L // NQH              # 1024
NKP = LT // 2               # 8 k-tile pairs
SCALE = 1.0 / np.sqrt(np.float32(D))  # 0.125

LN2 = float(np.log(2.0))
# Schraudolph fp16: bitcast_fp16(int16(x*A + B)).  B centered so the
# mean multiplicative error is ~1 (HW float->int16 truncates).  The
# -1024 makes e_b1 = S_0(x)/2 so the vr / sqrt2*vr weighted pair sums
# to unit weight.
A16 = (1024.0 / LN2) * 0.125
B16 = 15360.0 - 58.41 - 1024.0
SQRT2 = float(np.sqrt(2.0))

_cached = {}


def _build():
    import concourse.bacc as bacc
    import concourse.tile as tile
    from concourse import mybir

    f32 = mybir.dt.float32
    fp16 = mybir.dt.float16
    i16 = mybir.dt.int16
    Exp = mybir.ActivationFunctionType.Exp
    Copy = mybir.ActivationFunctionType.Copy

    nc = bacc.Bacc("TRN2", target_bir_lowering=False, debug=False)

    q_d = nc.dram_tensor("query", [B_SH, L, D], f32, kind="ExternalInput")
    k_d = nc.dram_tensor("key", [B_SH, L, D], f32, kind="ExternalInput")
    v_d = nc.dram_tensor("value", [B_SH, L, D], f32, kind="ExternalInput")
    o_d = nc.dram_tensor("out", [B_SH, L, D], f32, kind="ExternalOutput")

    with tile.TileContext(nc) as tc:
        with (
            tc.tile_pool(name="consts", bufs=1) as consts,
            tc.tile_pool(name="nat", bufs=2) as nat,
            tc.tile_pool(name="nath", bufs=2) as nath,
            tc.tile_pool(name="tp2", bufs=2) as tp2,
            tc.tile_pool(name="qf", bufs=2) as qfp,
            tc.tile_pool(name="vr", bufs=2) as vrp,
            tc.tile_pool(name="er", bufs=8) as erp,
            tc.tile_pool(name="er2", bufs=4) as e2p,
            tc.tile_pool(name="pvsb", bufs=2) as pvsbp,
            tc.tile_pool(name="ot", bufs=2) as otp,
            tc.tile_pool(name="oall", bufs=2) as oallp,
            tc.tile_pool(name="rz", bufs=2) as rzp,
            tc.tile_pool(name="sps", bufs=3, space="PSUM") as sps,
            tc.tile_pool(name="pvps", bufs=2, space="PSUM") as pvps,
        ):
            # engine warm-up: ACT table load, gpsimd ucode IRAM loads,
            # PE HAM warm -- all overlapped with the first batch's DMAs
            wsrc = consts.tile([128, 512], fp16)
            nc.vector.memset(wsrc, 1.0)
            dummy = consts.tile([128, 1], f32)
            nc.vector.memset(dummy, 0.0)
            nc.scalar.activation(out=dummy, in_=dummy, func=Exp, scale=1.0)
            gdum = consts.tile([128, 2], fp16)
            gdum32 = consts.tile([128, 2], f32)
            nc.gpsimd.memset(gdum, 1.0)
            nc.gpsimd.tensor_copy(out=gdum32, in_=gdum)
            nc.gpsimd.tensor_scalar_mul(out=gdum32, in0=gdum, scalar1=2.0)

            def warmer():
                wt = sps.tile([128, QHW], f32, tag="s", name="wt")
                nc.tensor.matmul(wt[:, 0:512], wsrc[:, 0:128], wsrc,
                                 start=True, stop=True, skip_group_check=True)

            # ---------- per-batch persistent tiles ----------
            khT2s = {}
            qfulls = {}
            v_rs = {}     # b -> (vr, vr_sq2)

            def prep_load(b, first=False):
                q_nat = nat.tile([128, LT, D], f32, tag="qnat")
                k_nat = nat.tile([128, LT, D], f32, tag="knat")
                v_nat = nat.tile([128, LT, D], f32, tag="vnat")
                k_r = k_d.ap()[b].rearrange("(t p) d -> p t d", p=128)
                q_r = q_d.ap()[b].rearrange("(t p) d -> p t d", p=128)
                H = LT // 2
                if first:
                    # split K/V (sync queue) from Q (scalar queue) so the
                    # two input chains' transfers run in parallel
                    nc.sync.dma_start(out=k_nat[:, 0:H, :], in_=k_r[:, 0:H, :])
                    nc.sync.dma_start(out=k_nat[:, H:LT, :], in_=k_r[:, H:LT, :])
                    nc.scalar.dma_start(out=q_nat[:, 0:H, :], in_=q_r[:, 0:H, :])
                    nc.scalar.dma_start(out=q_nat[:, H:LT, :],
                                        in_=q_r[:, H:LT, :])
                else:
                    # steady-state loads ride the gpsimd software-DGE queue,
                    # keeping the sync queue free for xbars/dups/stores
                    nc.gpsimd.dma_start(out=k_nat, in_=k_r)
                    nc.gpsimd.dma_start(out=q_nat, in_=q_r)
                (nc.sync if first else nc.gpsimd).dma_start(
                    out=v_nat, in_=v_d.ap()[b].rearrange("(t p) d -> p t d", p=128))
                return q_nat, k_nat, v_nat

            def prep_xbar(b, q_nat, k_nat, v_nat, first=False):
                # Steady state: fp16 casts on gpsimd (idle engine; off the
                # critical path).  Batch 0: the casts gate the first matmul,
                # so parallelize K on the DVE, Q on ScalarE (Copy), V on
                # gpsimd, and run the Q-side xbar chain from the scalar DMA
                # queue so the two input chains overlap.
                kh_nat = nath.tile([128, LT, D], fp16, tag="khn")
                qh_nat = nath.tile([128, LT, D], fp16, tag="qhn")
                khT2 = tp2.tile([128, NKP, 128], fp16, tag="khT2")
                qhT2 = tp2.tile([128, NKP, 128], fp16, tag="qhT2")
                qfull = qfp.tile([128, NKP, 2, 128], fp16, tag="qf")
                vr = vrp.tile([128, LT, 68], fp16, tag="vr")
                vr2 = vrp.tile([128, LT, 68], fp16, tag="vr2")
                H = LT // 2
                HM = NKP // 2

                def qdup(qd, qhT2, m0, m1):
                    ms = slice(m0, m1)
                    qd.dma_start(out=qfull[0:64, ms, 0, :],
                                 in_=qhT2[0:64, ms, :])
                    qd.dma_start(out=qfull[0:64, ms, 1, :],
                                 in_=qhT2[64:128, ms, :])
                    qd.dma_start(out=qfull[64:128, ms, 0, :],
                                 in_=qhT2[0:64, ms, :])
                    qd.dma_start(out=qfull[64:128, ms, 1, :],
                                 in_=qhT2[64:128, ms, :])

                if first:
                    # half-granular parallel chains: K via DVE+sync,
                    # Q via ScalarE + scalar DMA queue, V via the DVE
                    nc.vector.tensor_copy(out=kh_nat[:, 0:H, :],
                                          in_=k_nat[:, 0:H, :])
                    nc.sync.dma_start_transpose(out=khT2[:, 0:HM, :],
                                                in_=kh_nat[:, 0:H, :])
                    nc.scalar.activation(out=qh_nat[:, 0:H, :],
                                         in_=q_nat[:, 0:H, :], func=Copy)
                    nc.scalar.dma_start_transpose(out=qhT2[:, 0:HM, :],
                                                  in_=qh_nat[:, 0:H, :])
                    qdup(nc.scalar, qhT2, 0, HM)
                    nc.vector.tensor_copy(out=kh_nat[:, H:LT, :],
                                          in_=k_nat[:, H:LT, :])
                    nc.sync.dma_start_transpose(out=khT2[:, HM:NKP, :],
                                                in_=kh_nat[:, H:LT, :])
                    nc.scalar.activation(out=qh_nat[:, H:LT, :],
                                         in_=q_nat[:, H:LT, :], func=Copy)
                    nc.scalar.dma_start_transpose(out=qhT2[:, HM:NKP, :],
                                                  in_=qh_nat[:, H:LT, :])
                    nc.vector.tensor_copy(out=vr[:, :, 0:D], in_=v_nat)
                    nc.vector.memset(vr[:, :, D:D + 1], 1.0)
                    qdup(nc.sync, qhT2, HM, NKP)
                elif False:
                    pass
                else:
                    nc.gpsimd.tensor_copy(out=kh_nat, in_=k_nat)
                    nc.gpsimd.tensor_copy(out=qh_nat, in_=q_nat)
                    nc.gpsimd.tensor_copy(out=vr[:, :, 0:D], in_=v_nat)
                    nc.gpsimd.memset(vr[:, :, D:D + 1], 1.0)
                    nc.sync.dma_start_transpose(out=khT2, in_=kh_nat)
                    nc.sync.dma_start_transpose(out=qhT2, in_=qh_nat)
                    qdup(nc.sync, qhT2, 0, NKP)
                # full-tile scale (incl pad cols) keeps the DVE in 4x mode
                nc.vector.tensor_scalar_mul(out=vr2, in0=vr, scalar1=SQRT2)

                khT2s[b] = khT2
                qfulls[b] = qfull
                v_rs[b] = (vr, vr2)

            pending = []   # deferred output-path jobs (run ~1 per kp slot)

            # Manual scheduling: the Tile scheduler reorders instructions by
            # its own cost model, which breaks the score-pair adjacency and
            # the software-pipelined PV placement.  A monotonically rising
            # wait-until pseudo-timestamp per "slot" pins the static order
            # per engine to emission order (ties broken by priority).
            slot = [0.0]

            def tick(dt=1.0):
                slot[0] += dt
                tc.tile_set_cur_wait(slot[0])

            XW = 512   # cols of the odd tile handled by the DVE 2-phase
            PVDELAY = 2
            carries = []   # delayed kps' pv-block closures (cross-q-half)

            def main_qh(b, qh, mid_job=None, last_qh=False):
                khT2 = khT2s[b]
                qfull = qfulls[b]
                vr, vr2 = v_rs[b]
                q0 = qh * QHW
                tw = qh * (NKP // 2)

                pv = [pvps.tile([D + 1, 512], f32, tag="pv", name=f"pv{j}")
                      for j in range(2)]

                def make_pv_block(e_a, e_b1, e_b2, ka, kb, first, last,
                                  pv=pv, vr=vr, vr2=vr2):
                    def block():
                        nc.tensor.matmul(pv[0], vr[:, kb, 0:D + 1],
                                         e_b1[:, 0:512], start=first,
                                         stop=False)
                        nc.tensor.matmul(pv[0], vr2[:, kb, 0:D + 1],
                                         e_b2[:, 0:512], start=False,
                                         stop=False)
                        nc.tensor.matmul(pv[1], vr[:, kb, 0:D + 1],
                                         e_b1[:, 512:1024], start=first,
                                         stop=False)
                        for j in range(2):
                            js = slice(j * 512, (j + 1) * 512)
                            nc.tensor.matmul(pv[j], vr[:, ka, 0:D + 1],
                                             e_a[:, js], start=False, stop=last)
                    return block

                for kp in range(NKP):
                    tick()
                    if pending:
                        pending.pop(0)()
                    if kp == 4 and mid_job is not None:
                        mid_job()
                    ka, kb = 2 * kp, 2 * kp + 1
                    s_a = sps.tile([128, QHW], f32, tag="s", name="s_a")
                    s_b = sps.tile([128, QHW], f32, tag="s", name="s_b")
                    for j in range(2):
                        js = slice(j * 512, (j + 1) * 512)
                        qw = slice(tw + 2 * j, tw + 2 * (j + 1))
                        nc.tensor.matmul(s_a[:, js], khT2[0:64, kp, :],
                                         qfull[0:64, qw, :, :],
                                         start=True, stop=True)
                        nc.tensor.matmul(s_b[:, js], khT2[64:128, kp, :],
                                         qfull[64:128, qw, :, :],
                                         start=True, stop=True)
                    # PV block from PVDELAY kp slots ago (possibly the
                    # previous q-half's tail): exp inputs landed long ago,
                    # so the in-order PE queue never waits and q-half
                    # boundaries stay seamless
                    if len(carries) >= PVDELAY:
                        carries.pop(0)()
                    e_a = erp.tile([128, QHW], fp16, tag="e", name="e_a")
                    e_b1 = erp.tile([128, QHW], fp16, tag="e", name="e_b1")
                    e_b2 = e2p.tile([128, XW], fp16, tag="e2", name="e_b2")
                    # odd tile first on the ACT queue: s_b then releases at
                    # ~the same time as the DVE side, leaving s_a as the
                    # consistently-late bank -> the score pair dual-issues.
                    nc.scalar.activation(out=e_b1[:, XW:QHW],
                                         in_=s_b[:, XW:QHW], func=Exp,
                                         scale=float(SCALE))
                    nc.scalar.activation(out=e_a, in_=s_a, func=Exp,
                                         scale=float(SCALE))
                    nc.vector.tensor_scalar(
                        out=e_b1[:, 0:XW].bitcast(i16), in0=s_b[:, 0:XW],
                        scalar1=float(A16), scalar2=float(B16),
                        op0=mybir.AluOpType.mult, op1=mybir.AluOpType.add)
                    nc.vector.tensor_scalar_add(
                        out=e_b2.bitcast(i16), in0=e_b1[:, 0:XW].bitcast(i16),
                        scalar1=-512)
                    carries.append(make_pv_block(
                        e_a, e_b1, e_b2, ka, kb,
                        first=(kp == 0), last=(kp == NKP - 1)))

                # output path: deferred into the next q-half's kp slots
                state = {}

                def evac_job(pv=pv, state=state):
                    pv_sb = pvsbp.tile([96, QHW], fp16, tag="pvsb")
                    nc.scalar.activation(out=pv_sb[0:D + 1, 0:512], in_=pv[0],
                                         func=Copy)
                    nc.vector.tensor_copy(out=pv_sb[0:D + 1, 512:1024],
                                          in_=pv[1])
                    o_t = otp.tile([128, 8, 96], fp16, tag="ot")
                    nc.sync.dma_start_transpose(out=o_t, in_=pv_sb)
                    state["o_t"] = o_t

                def skip_job():
                    pass

                def out_job(b=b, q0=q0, state=state):
                    o_t = state.pop("o_t")
                    rz = rzp.tile([128, 8], f32, tag="rz")
                    nc.vector.reciprocal(out=rz, in_=o_t[:, :, D:D + 1])
                    o_all = oallp.tile([128, 8, D], f32, tag="oall")
                    nc.gpsimd.tensor_tensor(
                        out=o_all, in0=o_t[:, :, 0:D],
                        in1=rz.to_broadcast([128, 8, D]),
                        op=mybir.AluOpType.mult)
                    nc.sync.dma_start(
                        out=o_d.ap()[b, q0:q0 + QHW, :].rearrange(
                            "(m p) d -> p m d", p=128),
                        in_=o_all)
                if last_qh:
                    # final q-half: drain the output chain inline, j-split
                    # and pipelined, to shorten the kernel tail
                    for blk in carries:
                        tick()
                        blk()
                    carries.clear()
                    tick()
                    pv_sb = pvsbp.tile([96, QHW], fp16, tag="pvsb")
                    nc.scalar.activation(out=pv_sb[0:D + 1, 0:512], in_=pv[0],
                                         func=Copy)
                    nc.vector.tensor_copy(out=pv_sb[0:D + 1, 512:1024],
                                          in_=pv[1])
                    o_t = otp.tile([128, 8, 96], fp16, tag="ot")
                    rz = rzp.tile([128, 8], f32, tag="rz")
                    o_all = oallp.tile([128, 8, D], f32, tag="oall")
                    for h in range(2):
                        hm = slice(h * 4, (h + 1) * 4)
                        nc.sync.dma_start_transpose(
                            out=o_t[:, hm, :], in_=pv_sb[:, h * 512:(h + 1) * 512])
                        nc.vector.reciprocal(out=rz[:, hm],
                                             in_=o_t[:, hm, D:D + 1])
                        nc.vector.tensor_tensor(
                            out=o_all[:, hm, :], in0=o_t[:, hm, 0:D],
                            in1=rz[:, hm].to_broadcast([128, 4, D]),
                            op=mybir.AluOpType.mult)
                        nc.sync.dma_start(
                            out=o_d.ap()[b, q0 + h * 512:q0 + (h + 1) * 512,
                                         :].rearrange("(m p) d -> p m d", p=128),
                            in_=o_all[:, hm, :])
                else:
                    # evac must follow the final trailing pv block of this
                    # q-half, which drains at kp1-2 of the next q-half
                    pending.extend([skip_job, skip_job, evac_job, skip_job,
                                    out_job])

            # ---------- schedule ----------
            stage0 = prep_load(0, first=True)
            for _ in range(60):
                tick(0.25)
                warmer()
            prep_xbar(0, *stage0, first=True)
            stages = {}
            for b in range(B_SH):
                if b + 1 < B_SH:
                    stages[b + 1] = prep_load(b + 1)
                    mid = (lambda b=b: prep_xbar(b + 1, *stages.pop(b + 1)))
                else:
                    mid = None
                main_qh(b, 0, mid_job=mid)
                main_qh(b, 1, last_qh=(b == B_SH - 1))
                khT2s.pop(b, None)
                qfulls.pop(b, None)
                v_rs.pop(b, None)
            for blk in carries:
                tick()
                blk()
            carries.clear()
            for job in pending:
                tick()
                job()

    nc.finalize()
    return nc


def _get_nc():
    if "nc" not in _cached:
        _cached["nc"] = _build()
    return _cached["nc"]


def kernel(query, key, value):
    from concourse.bass_utils import run_bass_kernel_spmd

    nc = _get_nc()
    query = np.ascontiguousarray(query, dtype=np.float32)
    key = np.ascontiguousarray(key, dtype=np.float32)
    value = np.ascontiguousarray(value, dtype=np.float32)

    in_maps = []
    for c in range(NCORES):
        sl = slice(c * B_SH, (c + 1) * B_SH)
        in_maps.append({
            "query": query[sl], "key": key[sl], "value": value[sl]})

    res = run_bass_kernel_spmd(nc, in_maps, core_ids=list(range(NCORES)))
    out = np.concatenate([r["out"] for r in res.results], axis=0)
    return out


# revision 54
# speedup vs baseline: 1.0508x; 1.0508x over previous
"""Dense dot-product attention on 8 Trainium2 NeuronCores.

Problem: query/key/value [32, 2048, 64] fp32 -> softmax(Q K^T / 8) V.
Sharding: batch dim split 4-per-core across 8 cores (data parallel, no
collectives). Each core computes full attention for its 4 batches.

v3 design: the PE does ONLY matmuls (no transposes); softmax exp is
split across ScalarE (true exp, even k-tiles) and the DVE (two-phase
Schraudolph bit-trick exp, odd k-tiles); all transposes ride the DMA
xbar.

Per-batch dataflow:
  1. DMA Q,K,V natural [2048,64] fp32 -> stage [128,16,64]; gpsimd
     casts to fp16 (Q,K natural; V into vr [128,16,68] with a ones
     column at 64 = PV denominator trick; vr_sq2 = sqrt(2)*vr).
  2. One xbar-transpose DMA for Q and K: [128,1024] fp16 ->
     [128,8,128] where out[p,m,c] = in[c, m*128+p]: partitions 0:64
     hold even k-tiles transposed, 64:128 odd k-tiles.  K is used in
     this paired layout directly; Q is unscrambled + duplicated into
     qfull [128, 8, 2, 128] (every q-tile on both partition halves).
  3. Scores: per k-pair, 2x2 matmuls with contraction 64 on opposite
     PE row-groups (tile_position (0,0)/(64,0) via base partitions);
     fp32 PSUM s_a (even tile) / s_b (odd tile) [128, 1024].
  4. exp (scale 1/8 folded): ScalarE activation on s_a -> e_a fp16.
     DVE on s_b: e_b1 = bitcast16(i16(A*x + B)) approximates exp/2
     with a +-3% log-linear sawtooth; e_b2 = e_b1 - 512 (bits) is the
     half-octave phase-shifted variant scaled 1/(2*sqrt2).  The PV
     matmul sums  vr^T e_b1 + (sqrt2 vr)^T e_b2  = exp * (1 + mean of
     the two sawtooth phases): ~0.45% rms residual, and softmax
     normalization cancels the common-mode bias.
  5. PV: lhsT = [V | 1] natural [128, 65] fp16 -> out^T [65, q] fp32
     PSUM accumulated over 16 k-tiles (6 matmuls per k-pair); row 64
     is the softmax denominator (the sqrt2 ones-column keeps the b2
     stream's denominator share consistent).
  6. ScalarE evacuates pv -> fp16 SBUF; xbar-transpose (issued from
     the scalar engine's DMA queue) -> o_t [128, 8, 96] with
     q = m*128+p; DVE reciprocal of col 64 + one broadcast
     tensor_tensor multiply normalizes -> o_all fp32; DMA store.
     The reciprocal/normalize/store for each q-half are emitted
     deferred, woven into the next q-half's instruction stream, so no
     engine queue head-of-line blocks on the output path.
"""

import numpy as np

B, L, D = 32, 2048, 64
NCORES = 8
B_SH = B // NCORES          # 4 batches per core
LT = L // 128               # 16 k/q tiles of 128
NQH = 2                     # q processed in halves of 1024
QHW = L // NQH              # 1024
NKP = LT // 2               # 8 k-tile pairs
SCALE = 1.0 / np.sqrt(np.float32(D))  # 0.125

LN2 = float(np.log(2.0))
# Schraudolph fp16: bitcast_fp16(int16(x*A + B)).  B centered so the
# mean multiplicative error is ~1 (HW float->int16 truncates).  The
# -1024 makes e_b1 = S_0(x)/2 so the vr / sqrt2*vr weighted pair sums
# to unit weight.
A16 = (1024.0 / LN2) * 0.125
B16 = 15360.0 - 58.41 - 1024.0
SQRT2 = float(np.sqrt(2.0))

_cached = {}


def _build():
    import concourse.bacc as bacc
    import concourse.tile as tile
    from concourse import mybir

    f32 = mybir.dt.float32
    fp16 = mybir.dt.float16
    i16 = mybir.dt.int16
    Exp = mybir.ActivationFunctionType.Exp
    Copy = mybir.ActivationFunctionType.Copy

    nc = bacc.Bacc("TRN2", target_bir_lowering=False, debug=False)

    q_d = nc.dram_tensor("query", [B_SH, L, D], f32, kind="ExternalInput")
    k_d = nc.dram_tensor("key", [B_SH, L, D], f32, kind="ExternalInput")
    v_d = nc.dram_tensor("value", [B_SH, L, D], f32, kind="ExternalInput")
    o_d = nc.dram_tensor("out", [B_SH, L, D], f32, kind="ExternalOutput")

    with tile.TileContext(nc) as tc:
        with (
            tc.tile_pool(name="consts", bufs=1) as consts,
            tc.tile_pool(name="nat", bufs=2) as nat,
            tc.tile_pool(name="nath", bufs=2) as nath,
            tc.tile_pool(name="tp2", bufs=2) as tp2,
            tc.tile_pool(name="qf", bufs=2) as qfp,
            tc.tile_pool(name="vr", bufs=2) as vrp,
            tc.tile_pool(name="er", bufs=8) as erp,
            tc.tile_pool(name="er2", bufs=4) as e2p,
            tc.tile_pool(name="pvsb", bufs=2) as pvsbp,
            tc.tile_pool(name="ot", bufs=2) as otp,
            tc.tile_pool(name="oall", bufs=2) as oallp,
            tc.tile_pool(name="rz", bufs=2) as rzp,
            tc.tile_pool(name="sps", bufs=3, space="PSUM") as sps,
            tc.tile_pool(name="pvps", bufs=2, space="PSUM") as pvps,
        ):
            # engine warm-up: ACT table load, gpsimd ucode IRAM loads,
            # PE HAM warm -- all overlapped with the first batch's DMAs
            wsrc = consts.tile([128, 512], fp16)
            nc.vector.memset(wsrc, 1.0)
            dummy = consts.tile([128, 1], f32)
            nc.vector.memset(dummy, 0.0)
            nc.scalar.activation(out=dummy, in_=dummy, func=Exp, scale=1.0)
            gdum = consts.tile([128, 2], fp16)
            gdum32 = consts.tile([128, 2], f32)
            nc.gpsimd.memset(gdum, 1.0)
            nc.gpsimd.tensor_copy(out=gdum32, in_=gdum)
            nc.gpsimd.tensor_scalar_mul(out=gdum32, in0=gdum, scalar1=2.0)

            def warmer():
                wt = sps.tile([128, QHW], f32, tag="s", name="wt")
                nc.tensor.matmul(wt[:, 0:512], wsrc[:, 0:128], wsrc,
                                 start=True, stop=True, skip_group_check=True)

            # ---------- per-batch persistent tiles ----------
            khT2s = {}
            qfulls = {}
            v_rs = {}     # b -> (vr, vr_sq2)

            def prep_load(b, first=False):
                q_nat = nat.tile([128, LT, D], f32, tag="qnat")
                k_nat = nat.tile([128, LT, D], f32, tag="knat")
                v_nat = nat.tile([128, LT, D], f32, tag="vnat")
                k_r = k_d.ap()[b].rearrange("(t p) d -> p t d", p=128)
                q_r = q_d.ap()[b].rearrange("(t p) d -> p t d", p=128)
                H = LT // 2
                if first:
                    # split K/V (sync queue) from Q (scalar queue) so the
                    # two input chains' transfers run in parallel
                    nc.sync.dma_start(out=k_nat[:, 0:H, :], in_=k_r[:, 0:H, :])
                    nc.sync.dma_start(out=k_nat[:, H:LT, :], in_=k_r[:, H:LT, :])
                    nc.scalar.dma_start(out=q_nat[:, 0:H, :], in_=q_r[:, 0:H, :])
                    nc.scalar.dma_start(out=q_nat[:, H:LT, :],
                                        in_=q_r[:, H:LT, :])
                else:
                    # steady-state loads ride the gpsimd software-DGE queue,
                    # keeping the sync queue free for xbars/dups/stores
                    nc.gpsimd.dma_start(out=k_nat, in_=k_r)
                    nc.gpsimd.dma_start(out=q_nat, in_=q_r)
                (nc.sync if first else nc.gpsimd).dma_start(
                    out=v_nat, in_=v_d.ap()[b].rearrange("(t p) d -> p t d", p=128))
                return q_nat, k_nat, v_nat

            def prep_xbar(b, q_nat, k_nat, v_nat, first=False):
                # Steady state: fp16 casts on gpsimd (idle engine; off the
                # critical path).  Batch 0: the casts gate the first matmul,
                # so parallelize K on the DVE, Q on ScalarE (Copy), V on
                # gpsimd, and run the Q-side xbar chain from the scalar DMA
                # queue so the two input chains overlap.
                kh_nat = nath.tile([128, LT, D], fp16, tag="khn")
                qh_nat = nath.tile([128, LT, D], fp16, tag="qhn")
                khT2 = tp2.tile([128, NKP, 128], fp16, tag="khT2")
                qhT2 = tp2.tile([128, NKP, 128], fp16, tag="qhT2")
                qfull = qfp.tile([128, NKP, 2, 128], fp16, tag="qf")
                vr = vrp.tile([128, LT, 68], fp16, tag="vr")
                vr2 = vrp.tile([128, LT, 68], fp16, tag="vr2")
                H = LT // 2
                HM = NKP // 2

                def qdup(qd, qhT2, m0, m1):
                    ms = slice(m0, m1)
                    qd.dma_start(out=qfull[0:64, ms, 0, :],
                                 in_=qhT2[0:64, ms, :])
                    qd.dma_start(out=qfull[0:64, ms, 1, :],
                                 in_=qhT2[64:128, ms, :])
                    qd.dma_start(out=qfull[64:128, ms, 0, :],
                                 in_=qhT2[0:64, ms, :])
                    qd.dma_start(out=qfull[64:128, ms, 1, :],
                                 in_=qhT2[64:128, ms, :])

                if first:
                    # half-granular parallel chains: K via DVE+sync,
                    # Q via ScalarE + scalar DMA queue, V via the DVE
                    nc.vector.tensor_copy(out=kh_nat[:, 0:H, :],
                                          in_=k_nat[:, 0:H, :])
                    nc.sync.dma_start_transpose(out=khT2[:, 0:HM, :],
                                                in_=kh_nat[:, 0:H, :])
                    nc.scalar.activation(out=qh_nat[:, 0:H, :],
                                         in_=q_nat[:, 0:H, :], func=Copy)
                    nc.scalar.dma_start_transpose(out=qhT2[:, 0:HM, :],
                                                  in_=qh_nat[:, 0:H, :])
                    qdup(nc.scalar, qhT2, 0, HM)
                    nc.vector.tensor_copy(out=kh_nat[:, H:LT, :],
                                          in_=k_nat[:, H:LT, :])
                    nc.sync.dma_start_transpose(out=khT2[:, HM:NKP, :],
                                                in_=kh_nat[:, H:LT, :])
                    nc.scalar.activation(out=qh_nat[:, H:LT, :],
                                         in_=q_nat[:, H:LT, :], func=Copy)
                    nc.scalar.dma_start_transpose(out=qhT2[:, HM:NKP, :],
                                                  in_=qh_nat[:, H:LT, :])
                    nc.vector.tensor_copy(out=vr[:, :, 0:D], in_=v_nat)
                    nc.vector.memset(vr[:, :, D:D + 1], 1.0)
                    qdup(nc.sync, qhT2, HM, NKP)
                elif False:
                    pass
                else:
                    nc.gpsimd.tensor_copy(out=kh_nat, in_=k_nat)
                    nc.gpsimd.tensor_copy(out=qh_nat, in_=q_nat)
                    nc.gpsimd.tensor_copy(out=vr[:, :, 0:D], in_=v_nat)
                    nc.gpsimd.memset(vr[:, :, D:D + 1], 1.0)
                    nc.sync.dma_start_transpose(out=khT2, in_=kh_nat)
                    nc.sync.dma_start_transpose(out=qhT2, in_=qh_nat)
                    qdup(nc.sync, qhT2, 0, NKP)
                # full-tile scale (incl pad cols) keeps the DVE in 4x mode
                nc.vector.tensor_scalar_mul(out=vr2, in0=vr, scalar1=SQRT2)

                khT2s[b] = khT2
                qfulls[b] = qfull
                v_rs[b] = (vr, vr2)

            pending = []   # deferred output-path jobs (run ~1 per kp slot)

            # Manual scheduling: the Tile scheduler reorders instructions by
            # its own cost model, which breaks the score-pair adjacency and
            # the software-pipelined PV placement.  A monotonically rising
            # wait-until pseudo-timestamp per "slot" pins the static order
            # per engine to emission order (ties broken by priority).
            slot = [0.0]

            def tick(dt=1.0):
                slot[0] += dt
                tc.tile_set_cur_wait(slot[0])

            XW = 512   # cols of the odd tile handled by the DVE 2-phase
            PVDELAY = 2
            carries = []   # delayed kps' pv-block closures (cross-q-half)

            def main_qh(b, qh, mid_job=None, last_qh=False):
                khT2 = khT2s[b]
                qfull = qfulls[b]
                vr, vr2 = v_rs[b]
                q0 = qh * QHW
                tw = qh * (NKP // 2)

                pv = [pvps.tile([D + 1, 512], f32, tag="pv", name=f"pv{j}")
                      for j in range(2)]

                def make_pv_block(e_a, e_b1, e_b2, ka, kb, first, last,
                                  pv=pv, vr=vr, vr2=vr2):
                    def block():
                        nc.tensor.matmul(pv[0], vr[:, kb, 0:D + 1],
                                         e_b1[:, 0:512], start=first,
                                         stop=False)
                        nc.tensor.matmul(pv[0], vr2[:, kb, 0:D + 1],
                                         e_b2[:, 0:512], start=False,
                                         stop=False)
                        nc.tensor.matmul(pv[1], vr[:, kb, 0:D + 1],
                                         e_b1[:, 512:1024], start=first,
                                         stop=False)
                        for j in range(2):
                            js = slice(j * 512, (j + 1) * 512)
                            nc.tensor.matmul(pv[j], vr[:, ka, 0:D + 1],
                                             e_a[:, js], start=False, stop=last)
                    return block

                for kp in range(NKP):
                    tick()
                    if pending:
                        pending.pop(0)()
                    if kp == 4 and mid_job is not None:
                        mid_job()
                    ka, kb = 2 * kp, 2 * kp + 1
                    s_a = sps.tile([128, QHW], f32, tag="s", name="s_a")
                    s_b = sps.tile([128, QHW], f32, tag="s", name="s_b")
                    for j in range(2):
                        js = slice(j * 512, (j + 1) * 512)
                        qw = slice(tw + 2 * j, tw + 2 * (j + 1))
                        nc.tensor.matmul(s_a[:, js], khT2[0:64, kp, :],
                                         qfull[0:64, qw, :, :],
                                         start=True, stop=True)
                        nc.tensor.matmul(s_b[:, js], khT2[64:128, kp, :],
                                         qfull[64:128, qw, :, :],
                                         start=True, stop=True)
                    # PV block from PVDELAY kp slots ago (possibly the
                    # previous q-half's tail): exp inputs landed long ago,
                    # so the in-order PE queue never waits and q-half
                    # boundaries stay seamless
                    if len(carries) >= PVDELAY:
                        carries.pop(0)()
                    e_a = erp.tile([128, QHW], fp16, tag="e", name="e_a")
                    e_b1 = erp.tile([128, QHW], fp16, tag="e", name="e_b1")
                    e_b2 = e2p.tile([128, XW], fp16, tag="e2", name="e_b2")
                    # odd tile first on the ACT queue: s_b then releases at
                    # ~the same time as the DVE side, leaving s_a as the
                    # consistently-late bank -> the score pair dual-issues.
                    nc.scalar.activation(out=e_b1[:, XW:QHW],
                                         in_=s_b[:, XW:QHW], func=Exp,
                                         scale=float(SCALE))
                    nc.scalar.activation(out=e_a, in_=s_a, func=Exp,
                                         scale=float(SCALE))
                    nc.vector.tensor_scalar(
                        out=e_b1[:, 0:XW].bitcast(i16), in0=s_b[:, 0:XW],
                        scalar1=float(A16), scalar2=float(B16),
                        op0=mybir.AluOpType.mult, op1=mybir.AluOpType.add)
                    nc.vector.tensor_scalar_add(
                        out=e_b2.bitcast(i16), in0=e_b1[:, 0:XW].bitcast(i16),
                        scalar1=-512)
                    carries.append(make_pv_block(
                        e_a, e_b1, e_b2, ka, kb,
                        first=(kp == 0), last=(kp == NKP - 1)))

                # output path: deferred into the next q-half's kp slots
                state = {}

                def evac_job(pv=pv, state=state):
                    pv_sb = pvsbp.tile([96, QHW], fp16, tag="pvsb")
                    nc.scalar.activation(out=pv_sb[0:D + 1, 0:512], in_=pv[0],
                                         func=Copy)
                    nc.vector.tensor_copy(out=pv_sb[0:D + 1, 512:1024],
                                          in_=pv[1])
                    o_t = otp.tile([128, 8, 96], fp16, tag="ot")
                    nc.sync.dma_start_transpose(out=o_t, in_=pv_sb)
                    state["o_t"] = o_t

                def skip_job():
                    pass

                def out_job(b=b, q0=q0, state=state):
                    o_t = state.pop("o_t")
                    rz = rzp.tile([128, 8], f32, tag="rz")
                    nc.vector.reciprocal(out=rz, in_=o_t[:, :, D:D + 1])
                    o_all = oallp.tile([128, 8, D], f32, tag="oall")
                    nc.gpsimd.tensor_tensor(
                        out=o_all, in0=o_t[:, :, 0:D],
                        in1=rz.to_broadcast([128, 8, D]),
                        op=mybir.AluOpType.mult)
                    nc.sync.dma_start(
                        out=o_d.ap()[b, q0:q0 + QHW, :].rearrange(
                            "(m p) d -> p m d", p=128),
                        in_=o_all)
                if last_qh:
                    # final q-half: drain the output chain inline, j-split
                    # and pipelined, to shorten the kernel tail
                    for blk in carries:
                        tick()
                        blk()
                    carries.clear()
                    tick()
                    pv_sb = pvsbp.tile([96, QHW], fp16, tag="pvsb")
                    nc.scalar.activation(out=pv_sb[0:D + 1, 0:512], in_=pv[0],
                                         func=Copy)
                    nc.vector.tensor_copy(out=pv_sb[0:D + 1, 512:1024],
                                          in_=pv[1])
                    o_t = otp.tile([128, 8, 96], fp16, tag="ot")
                    rz = rzp.tile([128, 8], f32, tag="rz")
                    o_all = oallp.tile([128, 8, D], f32, tag="oall")
                    for h in range(2):
                        hm = slice(h * 4, (h + 1) * 4)
                        nc.sync.dma_start_transpose(
                            out=o_t[:, hm, :], in_=pv_sb[:, h * 512:(h + 1) * 512])
                        nc.vector.reciprocal(out=rz[:, hm],
                                             in_=o_t[:, hm, D:D + 1])
                        nc.vector.tensor_tensor(
                            out=o_all[:, hm, :], in0=o_t[:, hm, 0:D],
                            in1=rz[:, hm].to_broadcast([128, 4, D]),
                            op=mybir.AluOpType.mult)
                        nc.sync.dma_start(
                            out=o_d.ap()[b, q0 + h * 512:q0 + (h + 1) * 512,
                                         :].rearrange("(m p) d -> p m d", p=128),
                            in_=o_all[:, hm, :])
                else:
                    # evac must follow the final trailing pv block of this
                    # q-half, which drains at kp1-2 of the next q-half
                    pending.extend([skip_job, skip_job, evac_job, skip_job,
                                    out_job])

            # ---------- schedule ----------
            stage0 = prep_load(0, first=True)
            for _ in range(60):
                tick(0.25)
                warmer()
            prep_xbar(0, *stage0, first=True)
            stages = {}
            for b in range(B_SH):
                if b + 1 < B_SH:
                    stages[b + 1] = prep_load(b + 1)
                main_qh(b, 0)
                if b + 1 < B_SH:
                    prep_xbar(b + 1, *stages.pop(b + 1))
                main_qh(b, 1, last_qh=(b == B_SH - 1))
                khT2s.pop(b, None)
                qfulls.pop(b, None)
                v_rs.pop(b, None)
            for blk in carries:
                tick()
                blk()
            carries.clear()
            for job in pending:
                tick()
                job()

    nc.finalize()
    return nc


def _get_nc():
    if "nc" not in _cached:
        _cached["nc"] = _build()
    return _cached["nc"]


def kernel(query, key, value):
    from concourse.bass_utils import run_bass_kernel_spmd

    nc = _get_nc()
    query = np.ascontiguousarray(query, dtype=np.float32)
    key = np.ascontiguousarray(key, dtype=np.float32)
    value = np.ascontiguousarray(value, dtype=np.float32)

    in_maps = []
    for c in range(NCORES):
        sl = slice(c * B_SH, (c + 1) * B_SH)
        in_maps.append({
            "query": query[sl], "key": key[sl], "value": value[sl]})

    res = run_bass_kernel_spmd(nc, in_maps, core_ids=list(range(NCORES)))
    out = np.concatenate([r["out"] for r in res.results], axis=0)
    return out
